# revision 25
# baseline (speedup 1.0000x reference)
"""Trainium2 Bass kernel for nn_BCAModule (bilateral cross-attention).

Full inputs in, full outputs out. Internally sharded over 8 NeuronCores:
core c handles batch b = c // 4 and query rows 32*(c%4) .. 32*(c%4)+32
(N_loc = 4096 of the N = 16384 queries). Pooled K/V ([64, 1024]) is built
cooperatively: each core pools its own spatial quarter ([64, 256]) and the
4-core group all-gathers.

Math (BN folded host-side, the two stacked 1x1convs collapse to one affine):
  fx    = Ax @ x + cx          [64, N]   (queries, full res)
  fself = As @ x + cs -> pool  [64, M]   (values)
  fy    = Ay @ y + cy -> pool  [64, M]   (keys)
  simT  = fy^T-free layout: simT[m, n] = sum_c fy[c, m] fx[c, n]
  att   = exp(simT) / Z[n],  Z = col-sum (via ones-row in value matrix)
  fout  = fself_aug @ att      [65, N]  (row 64 = Z)
  out   = x + Au @ (fout/Z) + cu
"""

import numpy as np

B, CX, CM, H, W = 2, 720, 64, 128, 128
NCORES = 8
RB = 32               # image rows per core
NL = RB * W           # 4096 local queries
ML = (RB // 4) * (W // 4)   # 256 local pooled positions
M = 4 * ML            # 1024 pooled positions per batch
KP = 120              # contraction chunk for CX=720
KC = CX // KP         # 6
NT = 512              # n tile
NTN = NL // NT        # 8
MCH = 128             # m chunk
NMC = M // MCH        # 8

_CACHE = {}


def _build_nc(repeat=1, phases=(1, 2)):
    import os
    import concourse.bass as bass
    from concourse import bacc
    import concourse.mybir as mybir
    import concourse.tile as tile
    from concourse.masks import make_identity

    F32 = mybir.dt.float32
    F32R = mybir.dt.float32r
    AF = mybir.ActivationFunctionType
    ALU = mybir.AluOpType

    import bass_rust as _br
    from concourse.bass_interp import InstructionExecutor as _IE

    # The Tile scheduler's internal single-core sim cannot execute remote
    # (cross-core) DMA descs/triggers; treat them as no-ops there. The
    # MultiCoreSim correctness path (collective_state set) and real HW are
    # unaffected; actual cross-core ordering is enforced by semaphore waits
    # injected post-schedule (deferred_waits).
    if not getattr(_IE, "_p2p_sched_patch", False):
        _ob = _IE.visit_InstRemoteDMABroadcastDescs
        _ot = _IE.visit_InstTriggerDma

        def _vb(self, ins, *, reg_snapshot=None, __o=_ob):
            if self.collective_state is None:
                return
            return __o(self, ins, reg_snapshot=reg_snapshot)

        def _vt(self, ins, *, reg_snapshot=None, __o=_ot):
            if self.collective_state is None:
                return
            return __o(self, ins, reg_snapshot=reg_snapshot)

        _IE.visit_InstRemoteDMABroadcastDescs = _vb
        _IE.visit_InstTriggerDma = _vt
        _IE._p2p_sched_patch = True

    nc = bacc.Bacc(None)

    data_sem = nc.alloc_semaphore("p2p_data")
    lsem = nc.alloc_semaphore("p2p_lsem")
    credit_sem = nc.alloc_semaphore("p2p_credit")
    hello_sem = nc.alloc_semaphore("p2p_hello")
    misc_lsem = nc.alloc_semaphore("p2p_misc")

    xq_d = nc.dram_tensor("xq", [CX, NL], F32, kind="ExternalInput")
    yq_d = nc.dram_tensor("yq", [CM, NL], F32, kind="ExternalInput")
    wks_d = nc.dram_tensor("wks", [CX, 128], F32, kind="ExternalInput")
    wy_d = nc.dram_tensor("wy", [CM, CM], F32, kind="ExternalInput")
    wu_d = nc.dram_tensor("wu", [CM, CX], F32, kind="ExternalInput")
    bxs_d = nc.dram_tensor("bxs", [128, 1], F32, kind="ExternalInput")
    by_d = nc.dram_tensor("by", [CM, 1], F32, kind="ExternalInput")
    bu_d = nc.dram_tensor("bu", [CX, 1], F32, kind="ExternalInput")
    out_d = nc.dram_tensor("out", [CX, NL], F32, kind="ExternalOutput")

    with tile.TileContext(nc) as tc:
        with (
            tc.tile_pool(name="wpool", bufs=1) as wp,
            tc.tile_pool(name="xpool", bufs=KC) as xp,
            tc.tile_pool(name="persist", bufs=1) as pers,
            tc.tile_pool(name="dram", bufs=2, space="DRAM") as dp,
        ):
            # ---------------- weights / constants (once) ----------------
            wks_st = wp.tile([KP, KC * 128], F32, tag="wks_st")
            nc.sync.dma_start(
                wks_st[:].rearrange("p (k m) -> p k m", k=KC),
                wks_d[:].rearrange("(k p) m -> p k m", k=KC),
            )
            w_ks_t = wp.tile([KP, KC * 128], F32R, tag="w_ks")
            nc.vector.tensor_copy(w_ks_t[:], wks_st[:])
            w_ks = w_ks_t[:]

            wy_st = wp.tile([CM, CM], F32, tag="wy_st")
            nc.sync.dma_start(wy_st[:], wy_d[:])
            w_y_t = wp.tile([CM, CM], F32R, tag="w_y")
            nc.vector.tensor_copy(w_y_t[:], wy_st[:])
            w_y = w_y_t[:]

            wu_st = wp.tile([CM, CX], F32, tag="wu_st")
            nc.sync.dma_start(wu_st[:], wu_d[:])
            w_u_t = wp.tile([CM, CX], F32R, tag="w_u")
            nc.vector.tensor_copy(w_u_t[:], wu_st[:])
            w_u = w_u_t[:]

            bxs_sb = wp.tile([128, 1], F32, tag="bxs")
            nc.sync.dma_start(bxs_sb[:], bxs_d[:])
            by_sb = wp.tile([CM, 1], F32, tag="by")
            nc.sync.dma_start(by_sb[:], by_d[:])
            bu_sb = wp.tile([KP, KC], F32, tag="bu")
            nc.sync.dma_start(
                bu_sb[:].rearrange("p (k o) -> p k o", k=KC),
                bu_d[:].rearrange("(k p) o -> p k o", k=KC),
            )

            ident = wp.tile([128, 128], F32, tag="ident")
            make_identity(nc, ident[:])
            ones_f = wp.tile([128, 1], F32, tag="ones_f")
            nc.gpsimd.memset(ones_f[:], 1.0)

            # persistent across phases
            fxfs = pers.tile([128, NL], F32R, tag="fxfs")
            fy_full = pers.tile([CM, M], F32R, tag="fy_full")
            fsa = [
                pers.tile([MCH, 65], F32R, tag=f"fsa{mc}", name=f"fsa{mc}")
                for mc in range(NMC)
            ]

            for rep in range(repeat):
                g_in = dp.tile([ML, 128], F32, tag="g_in", name="g_in")
                g_out = dp.tile([4 * ML, 128], F32, tag="g_out", name="g_out")
                x_tiles = []

                # ============ phase 1: projections, pooling, gather ============
                with tc.tile_pool(name="p1sb", bufs=1) as p1:
                    # ---- fy = pool(Ay @ y) + cy; pool pass1 straight off psum ----
                    y_sb = p1.tile([CM, NL], F32, tag="y_sb")
                    nc.sync.dma_start(y_sb[:], yq_d[:])
                    y_rt = p1.tile([CM, NL], F32R, tag="y_rt")
                    nc.vector.tensor_copy(y_rt[:], y_sb[:])
                    y_r = y_rt[:]
                    fy_p1 = p1.tile([CM, RB * 32], F32, tag="pool_p1")
                    with tc.tile_pool(name="p1ps", bufs=2, space="PSUM") as psy:
                        for nt in range(NTN):
                            s = slice(nt * NT, (nt + 1) * NT)
                            ps = psy.tile([CM, NT], F32, tag="psy")
                            nc.tensor.matmul(
                                ps[:], w_y, y_r[:, s], start=True, stop=True
                            )
                            nc.vector.tensor_reduce(
                                fy_p1[:, nt * 128:(nt + 1) * 128],
                                ps[:].rearrange("p (a w) -> p a w", w=4),
                                axis=mybir.AxisListType.X, op=ALU.max,
                            )
                    fy_pool = p1.tile([CM, ML], F32, tag="fy_pool")
                    nc.vector.tensor_reduce(
                        fy_pool[:],
                        fy_p1[:].rearrange(
                            "p (hb hh wb) -> p hb wb hh", hb=RB // 4, hh=4),
                        axis=mybir.AxisListType.X, op=ALU.max,
                    )
                    nc.vector.tensor_scalar_add(fy_pool[:], fy_pool[:], by_sb[:])

                    # ---- fx | fself = [Ax; As] @ x + [cx; cs] ----
                    with tc.tile_pool(name="ppps", bufs=1, space="PSUM") as ppp:
                        pp = [
                            ppp.tile([128, NT], F32, tag=f"pp{nt}", name=f"pp{nt}")
                            for nt in range(NTN)
                        ]
                        for k in range(KC):
                            x_k = xp.tile([KP, NL], F32, tag="x", name=f"x{k}")
                            nc.sync.dma_start(x_k[:], xq_d[k * KP:(k + 1) * KP, :])
                            x_tiles.append(x_k)
                            x_r = p1.tile([KP, NL], F32R, tag="x_r", bufs=2)
                            nc.scalar.activation(x_r[:], x_k[:], AF.Copy)
                            for nt in range(NTN):
                                nc.tensor.matmul(
                                    pp[nt][:],
                                    w_ks[:, k * 128:(k + 1) * 128],
                                    x_r[:, nt * NT:(nt + 1) * NT],
                                    start=(k == 0),
                                    stop=(k == KC - 1),
                                )
                        for nt in range(NTN):
                            nc.scalar.activation(
                                fxfs[:, nt * NT:(nt + 1) * NT], pp[nt][:],
                                AF.Identity, bias=bxs_sb[:],
                            )

                    # ---- fself pooling ----
                    fs_p1 = p1.tile([CM, RB * 32], F32, tag="pool_p1")
                    nc.vector.tensor_reduce(
                        fs_p1[:],
                        fxfs[CM:128, :].bitcast(F32).rearrange(
                            "p (a w) -> p a w", w=4),
                        axis=mybir.AxisListType.X, op=ALU.max,
                    )
                    fs_pool = p1.tile([CM, ML], F32, tag="fs_pool")
                    nc.vector.tensor_reduce(
                        fs_pool[:],
                        fs_p1[:].rearrange(
                            "p (hb hh wb) -> p hb wb hh", hb=RB // 4, hh=4),
                        axis=mybir.AxisListType.X, op=ALU.max,
                    )

                    # ---- transpose pooled -> [128, 256] p2p block ----
                    with tc.tile_pool(name="ptps", bufs=2, space="PSUM") as pt:
                        gt = p1.tile([128, 256], F32, tag="gt")
                        gt_writers = []
                        for j in range(2):
                            tps = pt.tile([128, CM], F32, tag="pt")
                            nc.tensor.transpose(
                                tps[:], fs_pool[:, j * 128:(j + 1) * 128],
                                ident[0:CM, 0:CM],
                            )
                            gt_writers.append(nc.vector.tensor_copy(
                                gt[:, j * 128:j * 128 + CM], tps[:]))
                            tps2 = pt.tile([128, CM], F32, tag="pt")
                            nc.tensor.transpose(
                                tps2[:], fy_pool[:, j * 128:(j + 1) * 128],
                                ident[0:CM, 0:CM],
                            )
                            gt_writers.append(nc.vector.tensor_copy(
                                gt[:, j * 128 + CM:(j + 1) * 128], tps2[:]))

                        # ---- all-gather pooled K/V within the batch group ----
                        nc.sync.dma_start(
                            g_in[:].rearrange("(j p) c -> p j c", p=128),
                            gt[:].rearrange("p (j c) -> p j c", j=2),
                        )
                        if "nocc" not in phases:
                            nc.gpsimd.collective_compute(
                                "AllGather",
                                ALU.bypass,
                                replica_groups=[[0, 1, 2, 3], [4, 5, 6, 7]],
                                ins=[g_in[:].opt()],
                                outs=[g_out[:].opt()],
                            )

                        # ---- unpack gathered K/V ----
                        for mc in range(NMC):
                            gg = p1.tile([128, 128], F32, tag="gg", bufs=2)
                            nc.sync.dma_start(
                                gg[:], g_out[mc * 128:(mc + 1) * 128, :])
                            nc.vector.tensor_copy(fsa[mc][:, 0:CM], gg[:, 0:CM])
                            nc.vector.tensor_copy(fsa[mc][:, CM:65], ones_f[:])
                            tps3 = pt.tile([CM, 128], F32, tag="ptb")
                            nc.tensor.transpose(tps3[:], gg[:, CM:128], ident[:])
                            nc.vector.tensor_copy(
                                fy_full[:, mc * 128:(mc + 1) * 128], tps3[:])

                # ============ phase 2: attention + output ============
                with (
                    tc.tile_pool(name="p2sb", bufs=1) as p2,
                    tc.tile_pool(name="p2ps", bufs=1, space="PSUM") as p2p,
                ):
                    for nt in range(NTN):
                        ns = slice(nt * NT, (nt + 1) * NT)
                        et = p2.tile([128, NMC * NT], F32R, tag="et", bufs=2)
                        for pr in range(NMC // 2):
                            st = p2p.tile([128, 2 * NT], F32, tag="sim", bufs=2)
                            for j in range(2):
                                mc = 2 * pr + j
                                nc.tensor.matmul(
                                    st[:, j * NT:(j + 1) * NT],
                                    fy_full[:, mc * 128:(mc + 1) * 128],
                                    fxfs[0:CM, ns],
                                    start=True, stop=True,
                                )
                            nc.scalar.activation(
                                et[:, 2 * pr * NT:(2 * pr + 2) * NT], st[:],
                                AF.Exp,
                            )
                        fo = p2p.tile([65, NT], F32, tag="fo", bufs=2)
                        for mc in range(NMC):
                            nc.tensor.matmul(
                                fo[:], fsa[mc][:], et[:, mc * NT:(mc + 1) * NT],
                                start=(mc == 0), stop=(mc == NMC - 1),
                            )
                        rz = p2.tile([1, NT], F32, tag="rz", bufs=2)
                        nc.vector.reciprocal(rz[:], fo[CM:65, :])
                        rzb = p2.tile([CM, NT], F32, tag="rzb", bufs=2)
                        nc.sync.dma_start(
                            rzb[:], rz[:].unsqueeze(1).broadcast_to([1, CM, NT])
                        )
                        fout_sb = p2.tile([CM, NT], F32R, tag="fout", bufs=2)
                        nc.vector.tensor_tensor(
                            fout_sb[:], fo[0:CM, :], rzb[:], op=ALU.mult
                        )
                        for ot in range(KC):
                            up = p2p.tile([KP, NT], F32, tag="up", bufs=2)
                            nc.tensor.matmul(
                                up[:], w_u[:, ot * KP:(ot + 1) * KP], fout_sb[:],
                                start=True, stop=True,
                            )
                            ob = p2.tile([KP, NT], F32, tag="ob", bufs=3)
                            nc.vector.scalar_tensor_tensor(
                                ob[:], up[:], bu_sb[:, ot:ot + 1],
                                x_tiles[ot][:, ns],
                                op0=ALU.add, op1=ALU.add,
                            )
                            nc.sync.dma_start(
                                out_d[ot * KP:(ot + 1) * KP, ns], ob[:])

    nc.finalize()
    return nc


def _fold(W1, s1, b1, W2, s2, b2):
    W1 = W1.astype(np.float64)
    W2 = W2.astype(np.float64)
    A1 = s1.astype(np.float64)[:, None] * W1
    A2 = s2.astype(np.float64)[:, None] * W2
    A = A2 @ A1
    c = A2 @ b1.astype(np.float64) + b2.astype(np.float64)
    return A, c


def _get_runner():
    if "runner" in _CACHE:
        return _CACHE["runner"]

    import jax
    import jax.numpy as jnp
    from jax.sharding import Mesh, PartitionSpec
    from jax.experimental.shard_map import shard_map
    import concourse.bass as bass
    import concourse.mybir as mybir
    from concourse import bass2jax
    from concourse.bass2jax import _bass_exec_p, install_neuronx_cc_hook, partition_id_tensor

    nc = _build_nc()
    install_neuronx_cc_hook()

    partition_name = nc.partition_id_tensor.name if nc.partition_id_tensor else None
    in_names, out_names, out_avals, zero_shapes = [], [], [], []
    for alloc in nc.m.functions[0].allocations:
        if not isinstance(alloc, mybir.MemoryLocationSet):
            continue
        if getattr(alloc, "kind", None) == "ExternalInput":
            name = alloc.memorylocations[0].name
            if name != partition_name:
                in_names.append(name)
        elif getattr(alloc, "kind", None) == "ExternalOutput":
            name = alloc.memorylocations[0].name
            out_names.append(name)
            shape = tuple(alloc.tensor_shape)
            dtype = mybir.dt.np(alloc.dtype)
            out_avals.append(jax.core.ShapedArray(shape, dtype))
            zero_shapes.append((shape, dtype))

    n_params = len(in_names)
    n_outs = len(out_avals)
    all_in_names = list(in_names) + list(out_names)
    if partition_name is not None:
        all_in_names.append(partition_name)

    def _body(*args):
        operands = list(args)
        if partition_name is not None:
            operands.append(partition_id_tensor())
        outs = _bass_exec_p.bind(
            *operands,
            out_avals=tuple(out_avals),
            in_names=tuple(all_in_names),
            out_names=tuple(out_names),
            lowering_input_output_aliases=(),
            sim_require_finite=True,
            sim_require_nnan=True,
            nc=nc,
        )
        return tuple(outs)

    devices = jax.devices()[:NCORES]
    mesh = Mesh(np.asarray(devices), ("core",))
    in_specs = (PartitionSpec("core"),) * (n_params + n_outs)
    out_specs = (PartitionSpec("core"),) * n_outs
    donate = tuple(range(n_params, n_params + n_outs))
    sharded = jax.jit(
        shard_map(_body, mesh=mesh, in_specs=in_specs, out_specs=out_specs,
                  check_rep=False),
        donate_argnums=donate,
        keep_unused=True,
    )

    runner = {
        "sharded": sharded,
        "in_names": in_names,
        "out_names": out_names,
        "zero_shapes": zero_shapes,
        "n_params": n_params,
    }
    _CACHE["runner"] = runner
    return runner


def _prep_in_maps(inputs):
    x = np.ascontiguousarray(inputs["x"], dtype=np.float32)
    y = np.ascontiguousarray(inputs["y"], dtype=np.float32)

    Ax, cx = _fold(inputs["Wx1"], inputs["sx1"], inputs["bx1"],
                   inputs["Wx2"], inputs["sx2"], inputs["bx2"])
    As, cs = _fold(inputs["Ws1"], inputs["ss1"], inputs["bs1"],
                   inputs["Ws2"], inputs["ss2"], inputs["bs2"])
    Ay, cy = _fold(inputs["Wy1"], inputs["sy1"], inputs["by1"],
                   inputs["Wy2"], inputs["sy2"], inputs["by2"])
    Au = inputs["su"].astype(np.float64)[:, None] * inputs["Wu"].astype(np.float64)
    cu = inputs["bu"].astype(np.float64)

    wks = np.ascontiguousarray(
        np.concatenate([Ax.T, As.T], axis=1), dtype=np.float32)   # [720, 128]
    wy = np.ascontiguousarray(Ay.T, dtype=np.float32)             # [64, 64]
    wu = np.ascontiguousarray(Au.T, dtype=np.float32)             # [64, 720]
    bxs = np.concatenate([cx, cs])[:, None].astype(np.float32)    # [128, 1]
    by = cy[:, None].astype(np.float32)
    bu = cu[:, None].astype(np.float32)

    in_maps = []
    for c in range(NCORES):
        b, r = divmod(c, 4)
        xq = np.ascontiguousarray(
            x[b, :, r * RB:(r + 1) * RB, :].reshape(CX, NL))
        yq = np.ascontiguousarray(
            y[b, :, r * RB:(r + 1) * RB, :].reshape(CM, NL))
        in_maps.append({
            "xq": xq, "yq": yq, "wks": wks, "wy": wy, "wu": wu,
            "bxs": bxs, "by": by, "bu": bu,
        })
    return in_maps


def _run(in_maps):
    r = _get_runner()
    concat_in = [
        np.concatenate([in_maps[c][name] for c in range(NCORES)], axis=0)
        for name in r["in_names"]
    ]
    concat_zeros = [
        np.zeros((NCORES * s[0], *s[1:]), dt) for (s, dt) in r["zero_shapes"]
    ]
    out_arrs = r["sharded"](*concat_in, *concat_zeros)
    outs = []
    for i, name in enumerate(r["out_names"]):
        arr = np.asarray(out_arrs[i])
        outs.append(arr.reshape(NCORES, -1, arr.shape[-1]))
    return {name: outs[i] for i, name in enumerate(r["out_names"])}


def kernel(**inputs):
    in_maps = _prep_in_maps(inputs)
    res = _run(in_maps)
    o = res["out"]  # [8, 720, 4096]
    out = np.empty((B, CX, H, W), dtype=np.float32)
    for c in range(NCORES):
        b, r = divmod(c, 4)
        out[b, :, r * RB:(r + 1) * RB, :] = o[c].reshape(CX, RB, W)
    return out


# revision 27
# speedup vs baseline: 40737.1401x; 40737.1401x over previous
"""Trainium2 Bass kernel for nn_BCAModule (bilateral cross-attention).

Full inputs in, full outputs out. Internally sharded over 8 NeuronCores:
core c handles batch b = c // 4 and query rows 32*(c%4) .. 32*(c%4)+32
(N_loc = 4096 of the N = 16384 queries). Pooled K/V ([64, 1024]) is built
cooperatively: each core pools its own spatial quarter ([64, 256]) and the
4-core group all-gathers.

Math (BN folded host-side, the two stacked 1x1convs collapse to one affine):
  fx    = Ax @ x + cx          [64, N]   (queries, full res)
  fself = As @ x + cs -> pool  [64, M]   (values)
  fy    = Ay @ y + cy -> pool  [64, M]   (keys)
  simT  = fy^T-free layout: simT[m, n] = sum_c fy[c, m] fx[c, n]
  att   = exp(simT) / Z[n],  Z = col-sum (via ones-row in value matrix)
  fout  = fself_aug @ att      [65, N]  (row 64 = Z)
  out   = x + Au @ (fout/Z) + cu
"""

import numpy as np

B, CX, CM, H, W = 2, 720, 64, 128, 128
NCORES = 8
RB = 32               # image rows per core
NL = RB * W           # 4096 local queries
ML = (RB // 4) * (W // 4)   # 256 local pooled positions
M = 4 * ML            # 1024 pooled positions per batch
KP = 120              # contraction chunk for CX=720
KC = CX // KP         # 6
NT = 512              # n tile
NTN = NL // NT        # 8
MCH = 128             # m chunk
NMC = M // MCH        # 8

_CACHE = {}


def _build_nc(repeat=1, phases=(1, 2)):
    import os
    import concourse.bass as bass
    from concourse import bacc
    import concourse.mybir as mybir
    import concourse.tile as tile
    from concourse.masks import make_identity

    F32 = mybir.dt.float32
    F32R = mybir.dt.float32r
    AF = mybir.ActivationFunctionType
    ALU = mybir.AluOpType

    import bass_rust as _br
    from concourse.bass_interp import InstructionExecutor as _IE

    # The Tile scheduler's internal single-core sim cannot execute remote
    # (cross-core) DMA descs/triggers; treat them as no-ops there. The
    # MultiCoreSim correctness path (collective_state set) and real HW are
    # unaffected; actual cross-core ordering is enforced by semaphore waits
    # injected post-schedule (deferred_waits).
    if not getattr(_IE, "_p2p_sched_patch", False):
        _ob = _IE.visit_InstRemoteDMABroadcastDescs
        _ot = _IE.visit_InstTriggerDma

        def _vb(self, ins, *, reg_snapshot=None, __o=_ob):
            if self.collective_state is None:
                return
            return __o(self, ins, reg_snapshot=reg_snapshot)

        def _vt(self, ins, *, reg_snapshot=None, __o=_ot):
            if self.collective_state is None:
                return
            return __o(self, ins, reg_snapshot=reg_snapshot)

        _IE.visit_InstRemoteDMABroadcastDescs = _vb
        _IE.visit_InstTriggerDma = _vt
        _IE._p2p_sched_patch = True

    nc = bacc.Bacc(None)

    data_sem = nc.alloc_semaphore("p2p_data")
    lsem = nc.alloc_semaphore("p2p_lsem")
    credit_sem = nc.alloc_semaphore("p2p_credit")
    hello_sem = nc.alloc_semaphore("p2p_hello")
    misc_lsem = nc.alloc_semaphore("p2p_misc")

    xq_d = nc.dram_tensor("xq", [CX, NL], F32, kind="ExternalInput")
    yq_d = nc.dram_tensor("yq", [CM, NL], F32, kind="ExternalInput")
    wks_d = nc.dram_tensor("wks", [CX, 128], F32, kind="ExternalInput")
    wy_d = nc.dram_tensor("wy", [CM, CM], F32, kind="ExternalInput")
    wu_d = nc.dram_tensor("wu", [CM, CX], F32, kind="ExternalInput")
    bxs_d = nc.dram_tensor("bxs", [128, 1], F32, kind="ExternalInput")
    by_d = nc.dram_tensor("by", [CM, 1], F32, kind="ExternalInput")
    bu_d = nc.dram_tensor("bu", [CX, 1], F32, kind="ExternalInput")
    out_d = nc.dram_tensor("out", [CX, NL], F32, kind="ExternalOutput")

    with tile.TileContext(nc) as tc:
        with (
            tc.tile_pool(name="wpool", bufs=1) as wp,
            tc.tile_pool(name="xpool", bufs=KC) as xp,
            tc.tile_pool(name="persist", bufs=1) as pers,
            tc.tile_pool(name="dram", bufs=2, space="DRAM") as dp,
        ):
            # ---------------- weights / constants (once) ----------------
            wks_st = wp.tile([KP, KC * 128], F32, tag="wks_st")
            nc.sync.dma_start(
                wks_st[:].rearrange("p (k m) -> p k m", k=KC),
                wks_d[:].rearrange("(k p) m -> p k m", k=KC),
            )
            w_ks_t = wp.tile([KP, KC * 128], F32R, tag="w_ks")
            nc.vector.tensor_copy(w_ks_t[:], wks_st[:])
            w_ks = w_ks_t[:]

            wy_st = wp.tile([CM, CM], F32, tag="wy_st")
            nc.sync.dma_start(wy_st[:], wy_d[:])
            w_y_t = wp.tile([CM, CM], F32R, tag="w_y")
            nc.vector.tensor_copy(w_y_t[:], wy_st[:])
            w_y = w_y_t[:]

            wu_st = wp.tile([CM, CX], F32, tag="wu_st")
            nc.sync.dma_start(wu_st[:], wu_d[:])
            w_u_t = wp.tile([CM, CX], F32R, tag="w_u")
            nc.vector.tensor_copy(w_u_t[:], wu_st[:])
            w_u = w_u_t[:]

            bxs_sb = wp.tile([128, 1], F32, tag="bxs")
            nc.sync.dma_start(bxs_sb[:], bxs_d[:])
            by_sb = wp.tile([CM, 1], F32, tag="by")
            nc.sync.dma_start(by_sb[:], by_d[:])
            bu_sb = wp.tile([KP, KC], F32, tag="bu")
            nc.sync.dma_start(
                bu_sb[:].rearrange("p (k o) -> p k o", k=KC),
                bu_d[:].rearrange("(k p) o -> p k o", k=KC),
            )

            ident = wp.tile([128, 128], F32, tag="ident")
            make_identity(nc, ident[:])
            ones_f = wp.tile([128, 1], F32, tag="ones_f")
            nc.gpsimd.memset(ones_f[:], 1.0)

            # persistent across phases
            fxfs = pers.tile([128, NL], F32R, tag="fxfs")
            fy_full = pers.tile([CM, M], F32R, tag="fy_full")
            fsa = [
                pers.tile([MCH, 65], F32R, tag=f"fsa{mc}", name=f"fsa{mc}")
                for mc in range(NMC)
            ]

            for rep in range(repeat):
                g_in = dp.tile([ML, 128], F32, tag="g_in", name="g_in")
                g_out = dp.tile([4 * ML, 128], F32, tag="g_out", name="g_out")
                x_tiles = []

                # ============ phase 1: projections, pooling, gather ============
                with tc.tile_pool(name="p1sb", bufs=1) as p1:
                    # ---- fy = pool(Ay @ y) + cy; pool pass1 straight off psum ----
                    y_sb = p1.tile([CM, NL], F32, tag="y_sb")
                    nc.sync.dma_start(y_sb[:], yq_d[:])
                    y_rt = p1.tile([CM, NL], F32R, tag="y_rt")
                    nc.vector.tensor_copy(y_rt[:], y_sb[:])
                    y_r = y_rt[:]
                    fy_p1 = p1.tile([CM, RB * 32], F32, tag="pool_p1")
                    with tc.tile_pool(name="p1ps", bufs=2, space="PSUM") as psy:
                        for nt in range(NTN):
                            s = slice(nt * NT, (nt + 1) * NT)
                            ps = psy.tile([CM, NT], F32, tag="psy")
                            nc.tensor.matmul(
                                ps[:], w_y, y_r[:, s], start=True, stop=True
                            )
                            nc.vector.tensor_reduce(
                                fy_p1[:, nt * 128:(nt + 1) * 128],
                                ps[:].rearrange("p (a w) -> p a w", w=4),
                                axis=mybir.AxisListType.X, op=ALU.max,
                            )
                    fy_pool = p1.tile([CM, ML], F32, tag="fy_pool")
                    nc.vector.tensor_reduce(
                        fy_pool[:],
                        fy_p1[:].rearrange(
                            "p (hb hh wb) -> p hb wb hh", hb=RB // 4, hh=4),
                        axis=mybir.AxisListType.X, op=ALU.max,
                    )
                    nc.vector.tensor_scalar_add(fy_pool[:], fy_pool[:], by_sb[:])

                    # ---- fx | fself = [Ax; As] @ x + [cx; cs] ----
                    with tc.tile_pool(name="ppps", bufs=1, space="PSUM") as ppp:
                        pp = [
                            ppp.tile([128, NT], F32, tag=f"pp{nt}", name=f"pp{nt}")
                            for nt in range(NTN)
                        ]
                        for k in range(KC):
                            x_k = xp.tile([KP, NL], F32, tag="x", name=f"x{k}")
                            nc.sync.dma_start(x_k[:], xq_d[k * KP:(k + 1) * KP, :])
                            x_tiles.append(x_k)
                            x_r = p1.tile([KP, NL], F32R, tag="x_r", bufs=2)
                            nc.scalar.activation(x_r[:], x_k[:], AF.Copy)
                            for nt in range(NTN):
                                nc.tensor.matmul(
                                    pp[nt][:],
                                    w_ks[:, k * 128:(k + 1) * 128],
                                    x_r[:, nt * NT:(nt + 1) * NT],
                                    start=(k == 0),
                                    stop=(k == KC - 1),
                                )
                        for nt in range(NTN):
                            nc.scalar.activation(
                                fxfs[:, nt * NT:(nt + 1) * NT], pp[nt][:],
                                AF.Identity, bias=bxs_sb[:],
                            )

                    # ---- fself pooling ----
                    fs_p1 = p1.tile([CM, RB * 32], F32, tag="pool_p1")
                    nc.vector.tensor_reduce(
                        fs_p1[:],
                        fxfs[CM:128, :].bitcast(F32).rearrange(
                            "p (a w) -> p a w", w=4),
                        axis=mybir.AxisListType.X, op=ALU.max,
                    )
                    fs_pool = p1.tile([CM, ML], F32, tag="fs_pool")
                    nc.vector.tensor_reduce(
                        fs_pool[:],
                        fs_p1[:].rearrange(
                            "p (hb hh wb) -> p hb wb hh", hb=RB // 4, hh=4),
                        axis=mybir.AxisListType.X, op=ALU.max,
                    )

                    # ---- transpose pooled -> [128, 256] p2p block ----
                    with tc.tile_pool(name="ptps", bufs=2, space="PSUM") as pt:
                        gt = p1.tile([128, 256], F32, tag="gt")
                        gt_writers = []
                        for j in range(2):
                            tps = pt.tile([128, CM], F32, tag="pt")
                            nc.tensor.transpose(
                                tps[:], fs_pool[:, j * 128:(j + 1) * 128],
                                ident[0:CM, 0:CM],
                            )
                            gt_writers.append(nc.vector.tensor_copy(
                                gt[:, j * 128:j * 128 + CM], tps[:]))
                            tps2 = pt.tile([128, CM], F32, tag="pt")
                            nc.tensor.transpose(
                                tps2[:], fy_pool[:, j * 128:(j + 1) * 128],
                                ident[0:CM, 0:CM],
                            )
                            gt_writers.append(nc.vector.tensor_copy(
                                gt[:, j * 128 + CM:(j + 1) * 128], tps2[:]))

                        # ---- all-gather pooled K/V within the batch group ----
                        nc.sync.dma_start(
                            g_in[:].rearrange("(j p) c -> p j c", p=128),
                            gt[:].rearrange("p (j c) -> p j c", j=2),
                        )
                        if "nocc" not in phases:
                            nc.gpsimd.collective_compute(
                                "AllGather",
                                ALU.bypass,
                                replica_groups=[[0, 1, 2, 3], [4, 5, 6, 7]],
                                ins=[g_in[:].opt()],
                                outs=[g_out[:].opt()],
                            )

                        # ---- unpack gathered K/V ----
                        for mc in range(NMC):
                            gg = p1.tile([128, 128], F32, tag="gg", bufs=2)
                            nc.sync.dma_start(
                                gg[:], g_out[mc * 128:(mc + 1) * 128, :])
                            nc.vector.tensor_copy(fsa[mc][:, 0:CM], gg[:, 0:CM])
                            nc.vector.tensor_copy(fsa[mc][:, CM:65], ones_f[:])
                            tps3 = pt.tile([CM, 128], F32, tag="ptb")
                            nc.tensor.transpose(tps3[:], gg[:, CM:128], ident[:])
                            nc.vector.tensor_copy(
                                fy_full[:, mc * 128:(mc + 1) * 128], tps3[:])

                # ============ phase 2: attention + output ============
                with (
                    tc.tile_pool(name="p2sb", bufs=1) as p2,
                    tc.tile_pool(name="p2ps", bufs=1, space="PSUM") as p2p,
                ):
                    for nt in range(NTN):
                        ns = slice(nt * NT, (nt + 1) * NT)
                        et = p2.tile([128, NMC * NT], F32R, tag="et", bufs=2)
                        for pr in range(NMC // 2):
                            st = p2p.tile([128, 2 * NT], F32, tag="sim", bufs=2)
                            for j in range(2):
                                mc = 2 * pr + j
                                nc.tensor.matmul(
                                    st[:, j * NT:(j + 1) * NT],
                                    fy_full[:, mc * 128:(mc + 1) * 128],
                                    fxfs[0:CM, ns],
                                    start=True, stop=True,
                                )
                            nc.scalar.activation(
                                et[:, 2 * pr * NT:(2 * pr + 2) * NT], st[:],
                                AF.Exp,
                            )
                        fo = p2p.tile([65, NT], F32, tag="fo", bufs=2)
                        for mc in range(NMC):
                            nc.tensor.matmul(
                                fo[:], fsa[mc][:], et[:, mc * NT:(mc + 1) * NT],
                                start=(mc == 0), stop=(mc == NMC - 1),
                            )
                        rz = p2.tile([1, NT], F32, tag="rz", bufs=2)
                        nc.vector.reciprocal(rz[:], fo[CM:65, :])
                        rzb = p2.tile([CM, NT], F32, tag="rzb", bufs=2)
                        nc.sync.dma_start(
                            rzb[:], rz[:].unsqueeze(1).broadcast_to([1, CM, NT])
                        )
                        fout_sb = p2.tile([CM, NT], F32R, tag="fout", bufs=2)
                        nc.vector.tensor_tensor(
                            fout_sb[:], fo[0:CM, :], rzb[:], op=ALU.mult
                        )
                        for ot in range(KC):
                            up = p2p.tile([KP, NT], F32, tag="up", bufs=2)
                            nc.tensor.matmul(
                                up[:], w_u[:, ot * KP:(ot + 1) * KP], fout_sb[:],
                                start=True, stop=True,
                            )
                            ob = p2.tile([KP, NT], F32, tag="ob", bufs=3)
                            nc.vector.scalar_tensor_tensor(
                                ob[:], up[:], bu_sb[:, ot:ot + 1],
                                x_tiles[ot][:, ns],
                                op0=ALU.add, op1=ALU.add,
                            )
                            nc.sync.dma_start(
                                out_d[ot * KP:(ot + 1) * KP, ns], ob[:])

    nc.finalize()
    return nc


def _fold(W1, s1, b1, W2, s2, b2):
    W1 = W1.astype(np.float64)
    W2 = W2.astype(np.float64)
    A1 = s1.astype(np.float64)[:, None] * W1
    A2 = s2.astype(np.float64)[:, None] * W2
    A = A2 @ A1
    c = A2 @ b1.astype(np.float64) + b2.astype(np.float64)
    return A, c


def _get_runner():
    if "runner" in _CACHE:
        return _CACHE["runner"]

    import jax
    import jax.numpy as jnp
    from jax.sharding import Mesh, PartitionSpec
    from jax.experimental.shard_map import shard_map
    import concourse.bass as bass
    import concourse.mybir as mybir
    from concourse import bass2jax
    from concourse.bass2jax import _bass_exec_p, install_neuronx_cc_hook, partition_id_tensor

    nc = _build_nc()
    install_neuronx_cc_hook()

    partition_name = nc.partition_id_tensor.name if nc.partition_id_tensor else None
    in_names, out_names, out_avals, zero_shapes = [], [], [], []
    for alloc in nc.m.functions[0].allocations:
        if not isinstance(alloc, mybir.MemoryLocationSet):
            continue
        if getattr(alloc, "kind", None) == "ExternalInput":
            name = alloc.memorylocations[0].name
            if name != partition_name:
                in_names.append(name)
        elif getattr(alloc, "kind", None) == "ExternalOutput":
            name = alloc.memorylocations[0].name
            out_names.append(name)
            shape = tuple(alloc.tensor_shape)
            dtype = mybir.dt.np(alloc.dtype)
            out_avals.append(jax.core.ShapedArray(shape, dtype))
            zero_shapes.append((shape, dtype))

    n_params = len(in_names)
    n_outs = len(out_avals)
    all_in_names = list(in_names) + list(out_names)
    if partition_name is not None:
        all_in_names.append(partition_name)

    def _body(*args):
        operands = list(args)
        if partition_name is not None:
            operands.append(partition_id_tensor())
        outs = _bass_exec_p.bind(
            *operands,
            out_avals=tuple(out_avals),
            in_names=tuple(all_in_names),
            out_names=tuple(out_names),
            lowering_input_output_aliases=(),
            sim_require_finite=True,
            sim_require_nnan=True,
            nc=nc,
        )
        return tuple(outs)

    devices = jax.devices()[:NCORES]
    mesh = Mesh(np.asarray(devices), ("core",))
    in_specs = (PartitionSpec("core"),) * (n_params + n_outs)
    out_specs = (PartitionSpec("core"),) * n_outs
    sharded = jax.jit(
        shard_map(_body, mesh=mesh, in_specs=in_specs, out_specs=out_specs,
                  check_rep=False),
        keep_unused=True,
    )

    runner = {
        "sharded": sharded,
        "in_names": in_names,
        "out_names": out_names,
        "zero_shapes": zero_shapes,
        "n_params": n_params,
    }
    _CACHE["runner"] = runner
    return runner


def _prep_in_maps(inputs):
    x = np.ascontiguousarray(inputs["x"], dtype=np.float32)
    y = np.ascontiguousarray(inputs["y"], dtype=np.float32)

    Ax, cx = _fold(inputs["Wx1"], inputs["sx1"], inputs["bx1"],
                   inputs["Wx2"], inputs["sx2"], inputs["bx2"])
    As, cs = _fold(inputs["Ws1"], inputs["ss1"], inputs["bs1"],
                   inputs["Ws2"], inputs["ss2"], inputs["bs2"])
    Ay, cy = _fold(inputs["Wy1"], inputs["sy1"], inputs["by1"],
                   inputs["Wy2"], inputs["sy2"], inputs["by2"])
    Au = inputs["su"].astype(np.float64)[:, None] * inputs["Wu"].astype(np.float64)
    cu = inputs["bu"].astype(np.float64)

    wks = np.ascontiguousarray(
        np.concatenate([Ax.T, As.T], axis=1), dtype=np.float32)   # [720, 128]
    wy = np.ascontiguousarray(Ay.T, dtype=np.float32)             # [64, 64]
    wu = np.ascontiguousarray(Au.T, dtype=np.float32)             # [64, 720]
    bxs = np.concatenate([cx, cs])[:, None].astype(np.float32)    # [128, 1]
    by = cy[:, None].astype(np.float32)
    bu = cu[:, None].astype(np.float32)

    in_maps = []
    for c in range(NCORES):
        b, r = divmod(c, 4)
        xq = np.ascontiguousarray(
            x[b, :, r * RB:(r + 1) * RB, :].reshape(CX, NL))
        yq = np.ascontiguousarray(
            y[b, :, r * RB:(r + 1) * RB, :].reshape(CM, NL))
        in_maps.append({
            "xq": xq, "yq": yq, "wks": wks, "wy": wy, "wu": wu,
            "bxs": bxs, "by": by, "bu": bu,
        })
    return in_maps


def _run(in_maps):
    r = _get_runner()
    concat_in = [
        np.concatenate([in_maps[c][name] for c in range(NCORES)], axis=0)
        for name in r["in_names"]
    ]
    if "dz" not in _CACHE:
        import jax
        from jax.sharding import Mesh, PartitionSpec, NamedSharding
        mesh = Mesh(np.asarray(jax.devices()[:NCORES]), ("core",))
        sh = NamedSharding(mesh, PartitionSpec("core"))
        _CACHE["dz"] = [
            jax.device_put(np.zeros((NCORES * s[0], *s[1:]), dt), sh)
            for (s, dt) in r["zero_shapes"]
        ]
    out_arrs = r["sharded"](*concat_in, *_CACHE["dz"])
    outs = []
    for i, name in enumerate(r["out_names"]):
        arr = np.asarray(out_arrs[i])
        outs.append(arr.reshape(NCORES, -1, arr.shape[-1]))
    return {name: outs[i] for i, name in enumerate(r["out_names"])}


def kernel(**inputs):
    in_maps = _prep_in_maps(inputs)
    res = _run(in_maps)
    o = res["out"]  # [8, 720, 4096]
    out = np.empty((B, CX, H, W), dtype=np.float32)
    for c in range(NCORES):
        b, r = divmod(c, 4)
        out[b, :, r * RB:(r + 1) * RB, :] = o[c].reshape(CX, RB, W)
    return out
